# revision 1
# baseline (speedup 1.0000x reference)
"""Trainium2 Bass kernel: batched time-domain cross-correlation.

Computes, for each of 2048 (=64x32) independent pairs (fp32):
    out[g, l] = sum_k d1[g, k + l - 301] * d2[g, k],   l in [0, 603)

Algorithm: overlap-save block correlation in a half-shift (negacyclic)
real-DFT basis, so every matmul has a *shared* stationary operand (the
transform matrices) and batches all pairs in the moving operand:

  xp = d1 zero-padded/shifted; y = d2 zero-padded.
  out[B*c + j] = sum_v corr(w_{v+c}, y_v)[j]     (j in [0, B))
    w_s = xp[B*s : B*s + 2B]  (windows, stride B, length N=2B)
    y_v = y[B*v : B*v + B]    (blocks, zero-padded to N)
  Per-block circular corr via length-N negacyclic real DFT:
    bins k: Ur[k] = sum_n u[n] cos(pi n (2k+1)/N)
            Ui[k] = -sum_n u[n] sin(pi n (2k+1)/N),  k in [0, B)
    Z = X * conj(Y):  Zr = XrYr + XiYi ; Zi = XiYr - XrYi
    z[0:B] = Minv @ [Zr; Zi]  (exact: aliasing only corrupts j > B)

Mapping: forward transforms + inverse are PE matmuls with shared
stationaries; the pointwise spectral products run on the Vector engine
with the v-sum done by segmented tensor_reduce.

Sharding: data-parallel over the 2048 pairs, 256 pairs per core, 8 cores.
"""

import math
import os
import sys

import numpy as np

if "/opt/trn_rl_repo" not in sys.path:  # harness safety; axon site usually set
    sys.path.insert(0, "/opt/trn_rl_repo")

import concourse.bacc as bacc
import concourse.bass as bass
import concourse.mybir as mybir
import concourse.tile as tile
from concourse.bass_utils import run_bass_kernel_spmd

# ---- problem constants (hardcoded per contest contract) ----
NB_PAIRS, NCH, NT = 64, 32, 3000
LAGS = 603
SHIFT = 301  # NLAG + 1
NCORES = 8
G = (NB_PAIRS * NCH) // NCORES  # 256 pairs per core

# ---- tunables ----
B = int(os.environ.get("KB", "384"))  # lag/block granularity; N = 2B
GH = int(os.environ.get("KGH", "64"))  # pairs per g-chunk (SBUF working set)
USE_F32R = os.environ.get("KF32R", "1") == "1"  # full-rate matmuls (tf32-ish)
GP_FRAC = int(os.environ.get("KGP", "3"))  # every GP_FRAC-th TT stays on DVE
DT_MM = mybir.dt.float32r if USE_F32R else mybir.dt.float32
DT_Z = mybir.dt.float32r if USE_F32R else mybir.dt.float32
DT_VE = mybir.dt.float32  # vector-engine dtype

# derived
N = 2 * B
V = math.ceil(NT / B)  # y blocks
C = math.ceil(LAGS / B)  # output lag blocks
S = V + C - 1  # x windows
# fp32r ISA: innermost counts of matmul src/dst APs must be even
SP = S + (S % 2)  # padded window count (pad windows are all-zero)
CP = C + (C % 2)  # padded out-block count
assert V % 2 == 0, "y-block count must be even for fp32r"
BS = B // 128  # 128-chunks per B
NQ = N // 128  # contraction chunks of a full window
NJ = B // 128  # 128-chunks of B (bins halves / out j groups)
NR = 2 * NJ  # psum bin groups of the spectrum
U = (SP - 1) * BS + NQ  # 128-chunks in xp (covers padded windows)
NBB = U * 128
W = (V * B) // 128  # 128-chunks in y
GC = 512 // SP if SP > 4 else 128  # g per x-fwd column group
while GH % GC:
    GC -= 1
GCY = 512 // V
while GH % GCY:
    GCY -= 1
FG = 2 * GH  # g per inverse group (f = FG*CP in [256, 512] for CP in {2,4})
assert 256 <= FG * CP <= 512 and G % FG == 0

_PE_CACHE = {}
LAST_EXEC_NS = None
LAST_TRACE = None


def _matrices():
    n = np.arange(N, dtype=np.float64)[:, None]
    k = np.arange(B, dtype=np.float64)[None, :]
    theta = np.pi * n * (2 * k + 1) / N
    ffull = np.concatenate([np.cos(theta), -np.sin(theta)], axis=1)  # [N, 2B]
    minv = np.linalg.inv(ffull.T)[:B, :]  # [B, 2B]
    return ffull.astype(np.float32), minv.astype(np.float32)


def _const_tiles():
    """FW [128, NR*NQ*128]: FW[i, ((r*NQ)+q)*128 + col] = Ffull[128q+i, 128r+col]
    (r-major so each r's blocks are one contiguous DMA piece)
    MT [128, 3*NJ*NJ*128]: for zg in {Mr, Mi, -Mi}:
        MT[i, ((zg*NJ + rh)*NJ + jg)*128 + col] = M[128jg + col, 128rh + i]
    """
    ffull, minv = _matrices()
    fw = np.zeros((128, NR * NQ * 128), dtype=np.float32)
    for q in range(NQ):
        for r in range(NR):
            fw[:, (r * NQ + q) * 128 : (r * NQ + q + 1) * 128] = ffull[
                128 * q : 128 * (q + 1), 128 * r : 128 * (r + 1)
            ]
    mr = minv[:, :B]
    mi = minv[:, B:]
    mats = [mr, mi, -mi]
    mt = np.zeros((128, 3 * NJ * NJ * 128), dtype=np.float32)
    for zg in range(3):
        for rh in range(NJ):
            for jg in range(NJ):
                blk = mats[zg][128 * jg : 128 * (jg + 1), 128 * rh : 128 * (rh + 1)]
                base = ((zg * NJ + rh) * NJ + jg) * 128
                mt[:, base : base + 128] = blk.T
    return fw, mt


def build_kernel():
    nc = bacc.Bacc(
        "TRN2",
        target_bir_lowering=False,
        debug=False,
        num_devices=NCORES,
    )

    xp_d = nc.dram_tensor("xp", [128, G, U], DT_MM, kind="ExternalInput")
    yp_d = nc.dram_tensor("yp", [128, G, W], DT_MM, kind="ExternalInput")
    fw_d = nc.dram_tensor("fw", [128, NR * NQ * 128], DT_MM, kind="ExternalInput")
    mt_d = nc.dram_tensor("mt", [128, 3 * NJ * NJ * 128], DT_Z, kind="ExternalInput")
    out_d = nc.dram_tensor("out", [128, G, NJ, C], mybir.dt.float32,
                           kind="ExternalOutput")

    with tile.TileContext(nc, trace_sim=False) as tc:
        with (
            tc.tile_pool(name="const", bufs=1) as cpool,
            tc.tile_pool(name="io", bufs=2) as iopool,
            tc.tile_pool(name="spec", bufs=2) as spool,
            tc.tile_pool(name="work", bufs=3) as wpool,
            tc.tile_pool(name="zpool", bufs=1) as zpool,
            tc.tile_pool(name="psum", bufs=1, space=bass.MemorySpace.PSUM) as ppool,
        ):
            fw_t = cpool.tile([128, NR * NQ * 128], DT_MM, tag="fw")
            mt_t = cpool.tile([128, 3 * NJ * NJ * 128], DT_Z, tag="mt")
            zr = zpool.tile([128, NJ, G, CP], DT_Z, tag="zr")
            zi = zpool.tile([128, NJ, G, 2, CP], DT_Z, tag="zi")
            if CP > C:
                nc.gpsimd.memset(zr[:, :, :, C:], 0.0)
                nc.gpsimd.memset(zi[:, :, :, :, C:], 0.0)

            tt_i = 0

            def tt_eng():
                nonlocal tt_i
                e = nc.vector if tt_i % GP_FRAC == 0 else nc.gpsimd
                tt_i += 1
                return e

            outt = iopool.tile([128, G, NJ, C], mybir.dt.float32, tag="outt", bufs=1)
            for chunk in range(G // GH):
                g0 = chunk * GH
                xin = iopool.tile([128, GH, U], DT_MM, tag="xin", bufs=3)
                yin = iopool.tile([128, GH, W], DT_MM, tag="yin", bufs=3)
                nc.sync.dma_start(xin[:], xp_d.ap()[:, g0 : g0 + GH, :])
                nc.sync.dma_start(yin[:], yp_d.ap()[:, g0 : g0 + GH, :])
                if chunk == 1:
                    # mt is first needed by the inverse after chunk 1
                    nc.sync.dma_start(mt_t[:], mt_d.ap())
                if chunk == 0:
                    # consts after the first input tiles: r-pieces in use order
                    r_order0 = [x for rh in range(NJ) for x in (rh, NJ + rh)]
                    for r in r_order0:
                        nc.sync.dma_start(
                            fw_t[:, r * NQ * 128 : (r + 1) * NQ * 128],
                            fw_d.ap()[:, r * NQ * 128 : (r + 1) * NQ * 128],
                        )

                xs = spool.tile([128, NR, GH, SP], DT_VE, tag="xs")
                ys = spool.tile([128, NR, GH, V], DT_VE, tag="ys")

                # ---- forward transforms, x and y interleaved per bin
                # group; r-order pairs (rh, NJ+rh) so PW group rh unblocks
                # after two r-iterations
                r_order = [x for rh in range(NJ) for x in (rh, NJ + rh)]
                for r in r_order:
                    for cg in range(GH // GC):
                        ps = ppool.tile([128, GC, SP], mybir.dt.float32, tag="psA",
                                        bufs=4)
                        for q in range(NQ):
                            lhsT = fw_t[:, (r * NQ + q) * 128 : (r * NQ + q + 1) * 128]
                            rhs = xin[
                                :,
                                cg * GC : (cg + 1) * GC,
                                q : q + BS * (SP - 1) + 1 : BS,
                            ]
                            nc.tensor.matmul(
                                ps[:], lhsT, rhs, start=(q == 0), stop=(q == NQ - 1)
                            )
                        nc.scalar.copy(
                            out=xs[:, r, cg * GC : (cg + 1) * GC, :], in_=ps[:]
                        )
                    for cg in range(GH // GCY):
                        ps = ppool.tile([128, GCY, V], mybir.dt.float32, tag="psB",
                                        bufs=2)
                        for q in range(NJ):
                            lhsT = fw_t[:, (r * NQ + q) * 128 : (r * NQ + q + 1) * 128]
                            rhs = yin[
                                :,
                                cg * GCY : (cg + 1) * GCY,
                                q : q + BS * (V - 1) + 1 : BS,
                            ]
                            nc.tensor.matmul(
                                ps[:], lhsT, rhs, start=(q == 0), stop=(q == NJ - 1)
                            )
                        nc.scalar.copy(
                            out=ys[:, r, cg * GCY : (cg + 1) * GCY, :], in_=ps[:]
                        )

                # ---- pointwise products + v-sum (DVE + GpSimd) ----
                for c in range(C):
                    for rh in range(NJ):
                        pr = wpool.tile([128, GH, 2, V], DT_VE, tag="pr", bufs=4)
                        tt_eng().tensor_mul(
                            pr[:, :, 0, :],
                            xs[:, rh, :, c : c + V],
                            ys[:, rh, :, :],
                        )
                        tt_eng().tensor_mul(
                            pr[:, :, 1, :],
                            xs[:, NJ + rh, :, c : c + V],
                            ys[:, NJ + rh, :, :],
                        )
                        with nc.allow_low_precision(
                            "float32r output is 4-byte fp32 bits"
                        ):
                            nc.vector.tensor_reduce(
                                zr[:, rh, g0 : g0 + GH, c],
                                pr[:],
                                axis=mybir.AxisListType.XY,
                                op=mybir.AluOpType.add,
                            )
                        pr2 = wpool.tile([128, GH, 2, V], DT_VE, tag="pr", bufs=4)
                        tt_eng().tensor_mul(
                            pr2[:, :, 0, :],
                            xs[:, NJ + rh, :, c : c + V],
                            ys[:, rh, :, :],
                        )
                        tt_eng().tensor_mul(
                            pr2[:, :, 1, :],
                            xs[:, rh, :, c : c + V],
                            ys[:, NJ + rh, :, :],
                        )
                        with nc.allow_low_precision(
                            "float32r output is 4-byte fp32 bits"
                        ):
                            nc.vector.tensor_reduce(
                                zi[:, rh, g0 : g0 + GH, :, c],
                                pr2[:],
                                axis=mybir.AxisListType.X,
                                op=mybir.AluOpType.add,
                            )

                # ---- inverse transform for each completed pair-group ----
                if (chunk + 1) % (FG // GH) == 0:
                    fgi = chunk // (FG // GH)
                    gsl = slice(fgi * FG, (fgi + 1) * FG)
                    for jg in range(NJ):
                        ps = ppool.tile([128, FG, CP], mybir.dt.float32, tag="psC",
                                        bufs=2)
                        nmm = 3 * NJ
                        i = 0
                        for rh in range(NJ):
                            srcs = (
                                (0, zr[:, rh, gsl, :]),
                                (1, zi[:, rh, gsl, 0, :]),
                                (2, zi[:, rh, gsl, 1, :]),
                            )
                            for zg, rhs in srcs:
                                lhsT = mt_t[
                                    :,
                                    ((zg * NJ + rh) * NJ + jg) * 128 :
                                    ((zg * NJ + rh) * NJ + jg + 1) * 128,
                                ]
                                nc.tensor.matmul(
                                    ps[:], lhsT, rhs,
                                    start=(i == 0), stop=(i == nmm - 1),
                                )
                                i += 1
                        nc.scalar.copy(out=outt[:, gsl, jg, :], in_=ps[:, :, :C])

            nc.sync.dma_start(out_d.ap()[:], outt[:])

    nc.compile()
    return nc


def _prep_core_inputs(d1f, d2f, fw, mt, core):
    """d1f/d2f: [2048, 3000] fp32. Returns the in_map for `core`."""
    sl = slice(core * G, (core + 1) * G)
    x = d1f[sl]
    y = d2f[sl]
    xp = np.zeros((G, NBB), dtype=np.float32)
    xp[:, SHIFT : SHIFT + NT] = x
    yp = np.zeros((G, V * B), dtype=np.float32)
    yp[:, :NT] = y
    # device layouts: xpT[p, g, u] = xp[g, 128u + p]
    xpT = np.ascontiguousarray(xp.reshape(G, U, 128).transpose(2, 0, 1))
    ypT = np.ascontiguousarray(yp.reshape(G, W, 128).transpose(2, 0, 1))
    return {"xp": xpT, "yp": ypT, "fw": fw, "mt": mt}


def kernel(data1: np.ndarray, data2: np.ndarray) -> np.ndarray:
    import time

    d1f = np.ascontiguousarray(data1, dtype=np.float32).reshape(-1, NT)
    d2f = np.ascontiguousarray(data2, dtype=np.float32).reshape(-1, NT)
    fw, mt = _const_tiles()

    t0 = time.time()
    if "nc" not in _PE_CACHE:
        _PE_CACHE["nc"] = build_kernel()
    nc = _PE_CACHE["nc"]
    print(f"[kernel] build+compile {time.time() - t0:.1f}s", file=sys.stderr,
          flush=True)

    in_maps = [_prep_core_inputs(d1f, d2f, fw, mt, i) for i in range(NCORES)]
    t0 = time.time()
    res = run_bass_kernel_spmd(nc, in_maps, core_ids=list(range(NCORES)))
    print(f"[kernel] spmd run {time.time() - t0:.1f}s", file=sys.stderr, flush=True)
    global LAST_EXEC_NS, LAST_TRACE
    LAST_EXEC_NS = res.exec_time_ns
    LAST_TRACE = res.instructions_and_trace
    if res.exec_time_ns is not None:
        print(f"[kernel] HW exec {res.exec_time_ns} ns", file=sys.stderr, flush=True)

    outs = []
    for i in range(NCORES):
        o = res.results[i]["out"]  # [128, G, NJ, C]
        # out[g, B*c + 128*jg + p] = o[p, g, jg, c]
        full = o.transpose(1, 3, 2, 0).reshape(G, C * B)
        outs.append(full[:, :LAGS])
    return np.concatenate(outs, axis=0).reshape(NB_PAIRS, NCH, LAGS)

